# revision 48
# baseline (speedup 1.0000x reference)
"""Trainium2 Bass kernel for nn_ContinuousGenHyperConnections (v4).

Math per token t (row x of length 2048 = 4 streams of 512):
    xn = x / sqrt(mean(x^2) + eps)              (RMSNorm, folded into xt on host)
    F  = xn @ Wall^T + C                        (42 tiny projections, fused)
    sg = sigmoid(F[32:38]); dt affine; wr = sg[2:6]; ww = F[38:42]
    A  = dt_c*(M - M^T) - (dt_d/2)*R R^T,  M = F[0:16], R = F[16:32]
    u  = wr + wr @ A;  D = A + ww (x) u
    delta = D . h   (per-stream mixing);  out = x + delta

Device computes 256*delta only (fp8); the f32 residual add runs on host.

Key tricks vs the straightforward version:
- Block-diagonal mixing: the host uploads x fp8 tiles whose 128
  partitions are (stream j, token u) pairs per 32-token group; a 128x128
  block-diagonal lhsT computes 32 tokens x 4 streams x 512 features in
  ONE matmul -> 4 matmuls x 512 free rows per 128-token tile (the
  theoretical minimum) instead of 14.
- The bd lhsT needs D in (j,u)-partition layout; the token-major D is
  permuted on the PE with 16 tiny matmuls against identity slices
  (~zero cost in the free-dim cost model), then one gpsimd op applies
  the 32-token block mask, the 256x fp8 scale and the fp8 convert.
- The coefficient chain is batched over CB=4 tiles (tokens stay on
  partitions, the tile index becomes a free dim), amortizing per-op
  fixed costs (gpsimd Q7 launch, SBUF/PSUM access latency) 4x.
- Everything on the wire is fp8e4: x twice (token-major interleaved for
  mixing, d-major for the projection lhsT) plus 256*delta out.  Weights
  are prescaled by 32 into fp8 range; 1/32 is applied in the F drain.

Sharding: pure data parallel over B*T across 8 cores, params replicated.
"""

import os

import numpy as np
import ml_dtypes

import concourse.bacc as bacc
import concourse.tile as tile
from concourse import mybir
from concourse.bass_utils import run_bass_kernel_spmd

F32 = mybir.dt.float32
F16 = mybir.dt.float16
F8 = mybir.dt.float8e4
AF = mybir.ActivationFunctionType
OP = mybir.AluOpType
AX = mybir.AxisListType
NP_F8 = ml_dtypes.float8_e4m3

D = 2048
NSTR = 4
BS = 512
NF = 42            # 0:16 conv M | 16:32 diss R | 32 dt_c | 33 dt_d | 34:38 rd | 38:42 wr
P = 128
NCORES = 8
NBLK = D // P      # 16 d-blocks
MEGA = 4           # tiles per xT load (512 tokens)
WSCALE = 32.0      # fp8 weight prescale; 1/32 applied in the F drain
DSC = 256.0        # fp8 delta scale; host divides it back out
EPS = float(np.finfo(np.float32).eps)
DT_MIN, DT_MAX = 1e-3, 1.0

WARM_INIT = int(os.environ.get("K_WARM_INIT", "8"))   # initial PE ramp matmuls
WARM_TILE = int(os.environ.get("K_WARM_TILE", "2"))   # per-tile PE filler
CB = int(os.environ.get("K_CB", "4"))                 # chain batch (tiles)
WIDTHS = os.environ.get("K_WIDTHS", "")               # e.g. "2,2,4,4,4"
KA_POOL = os.environ.get("K_KA_POOL", "0") == "1"     # KA op on gpsimd

TRACE = False
LAST_RESULTS = None

_NC_CACHE = {}


def build_nc(tpc):
    assert tpc % (P * MEGA) == 0
    nt = tpc // P
    if WIDTHS:
        widths = [int(w) for w in WIDTHS.split(",")]
        assert sum(widths) <= nt
        while sum(widths) < nt:
            widths.append(min(CB, nt - sum(widths)))
    else:
        assert nt % CB == 0
        widths = [CB] * (nt // CB)
    starts = [sum(widths[:i]) for i in range(len(widths))]
    tile2blk = {}
    for B, (s0, w) in enumerate(zip(starts, widths)):
        for c in range(w):
            tile2blk[s0 + c] = (B, c, w)
    nc = bacc.Bacc("TRN2", target_bir_lowering=False)

    xh_in = nc.dram_tensor("xh", [tpc, D], F8, kind="ExternalInput")
    xt_in = nc.dram_tensor("xt", [P, NBLK, tpc], F8, kind="ExternalInput")
    wt_in = nc.dram_tensor("wt", [P, NBLK * NF], F8, kind="ExternalInput")
    # f32 consts packed: [0:42]=C  [42]=1/32  [43:75]=um
    cf_in = nc.dram_tensor("cf", [P, NF + 1 + 32], F32, kind="ExternalInput")
    # fp16 consts packed: [0:128]=eye  [128:384]=eyp(g,64)
    ch_in = nc.dram_tensor("ch", [P, P + 4 * 64], F16, kind="ExternalInput")
    dlt_out = nc.dram_tensor("dlt", [tpc, D], F8, kind="ExternalOutput")

    with tile.TileContext(nc) as tc:
        with (
            tc.tile_pool(name="consts", bufs=1) as consts,
            tc.tile_pool(name="xp", bufs=8) as xp,
            tc.tile_pool(name="xtp", bufs=3) as xtp,
            tc.tile_pool(name="bdp", bufs=3) as bdp,
            tc.tile_pool(name="dp", bufs=3) as dp,
            tc.tile_pool(name="small", bufs=3) as small,
            tc.tile_pool(name="warm_ps", bufs=1, space="PSUM") as warm_ps,
            tc.tile_pool(name="pj_ps", bufs=2, space="PSUM") as pj_ps,
            tc.tile_pool(name="v_ps", bufs=1, space="PSUM") as v_ps,
            tc.tile_pool(name="mx_ps", bufs=4, space="PSUM") as mx_ps,
        ):
            x_tiles = {}
            xt_megas = {}
            PF = 4

            def load_x(t):
                if t < nt:
                    xt_ = xp.tile([P, D], F8, name="x_t")
                    nc.sync.dma_start(out=xt_, in_=xh_in[t * P:(t + 1) * P, :])
                    x_tiles[t] = xt_

            def load_xt(m):
                if 0 <= m < nt // MEGA:
                    mt = xtp.tile([P, NBLK, MEGA * P], F8, name="xt_m")
                    nc.sync.dma_start(out=mt, in_=xt_in[:, :, m * MEGA * P:(m + 1) * MEGA * P])
                    xt_megas[m] = mt

            # DMA priority order: wt -> mega0 -> x0.. (tile 0's critical path)
            POS_CF = int(os.environ.get("K_POS_CF", "1"))
            POS_CH = int(os.environ.get("K_POS_CH", "2"))
            cf_s = consts.tile([P, NF + 1 + 32], F32)
            ch_s = consts.tile([P, P + 4 * 64], F16)

            def load_consts(pos):
                if POS_CF == pos:
                    nc.sync.dma_start(out=cf_s, in_=cf_in.ap())
                if POS_CH == pos:
                    nc.sync.dma_start(out=ch_s, in_=ch_in.ap())

            wt_s = consts.tile([P, NBLK, NF], F8)
            nc.sync.dma_start(out=wt_s, in_=wt_in.ap().rearrange("p (k f) -> p k f", k=NBLK))
            load_consts(0)
            load_xt(0)
            load_consts(1)
            load_x(0)
            load_consts(2)
            cv_s = cf_s[:, 0:NF]
            iv_s = cf_s[:, NF:NF + 1]
            um_s = cf_s[:, NF + 1:NF + 1 + 32]
            eye_s = ch_s[:, 0:P]
            eyp_s = ch_s[:, P:P + 4 * 64].rearrange("p (g c) -> p g c", g=4)
            for t in range(1, PF):
                load_x(t)
            load_consts(3)
            load_xt(1)

            # warm the PE p-state: big-free matmuls on the resident weight
            # tile keep the clock ramped so real matmuls run at full speed
            wm = warm_ps.tile([NF, BS], F32, tag="warm")
            wt_flat = wt_s.rearrange("p k f -> p (k f)")

            def warm(n):
                for _ in range(n):
                    nc.tensor.matmul(wm, lhsT=wt_s[:, 0, :], rhs=wt_flat[:, 0:BS],
                                     start=True, stop=True, skip_group_check=True)

            warm(WARM_INIT)

            state = {}
            blocks = {}

            def emit_proj(t):
                """fp8 projection matmuls for tile t (PE only), into the
                block-shared pj tile."""
                B, c, w = tile2blk[t]
                if c == 0:
                    blocks[B] = {"pj": pj_ps.tile([P, w, NF], F32, name="pj", tag="pj")}
                pj = blocks[B]["pj"]
                xm = xt_megas[t // MEGA]
                off = (t % MEGA) * P
                for k in range(NBLK):
                    nc.tensor.matmul(pj[:, c, :], lhsT=xm[:, k, off:off + P],
                                     rhs=wt_s[:, k, :],
                                     start=(k == 0), stop=(k == NBLK - 1))

            def emit_chain_a(B):
                """F drain + coefficient chain first half, batched over the
                block (tokens on partitions, tile index on a free dim)."""
                w = widths[B]
                bl = blocks[B]
                pj = bl.pop("pj")
                F4 = small.tile([P, w, NF], F32, name="F4")
                nc.vector.scalar_tensor_tensor(
                    out=F4, in0=pj, scalar=iv_s[:, 0:1],
                    in1=cv_s.unsqueeze(1).broadcast_to((P, w, NF)),
                    op0=OP.mult, op1=OP.add)
                bl["F4"] = F4
                SG = small.tile([P, w, 6], F32, name="SG")
                nc.scalar.activation(out=SG, in_=F4[:, :, 32:38], func=AF.Sigmoid)
                Fm = F4[:, :, 0:16].rearrange("p c (i j) -> p c i j", i=4)
                FmT = F4[:, :, 0:16].rearrange("p c (i j) -> p c j i", i=4)
                As = small.tile([P, w, 4, 4], F32, name="As")
                nc.gpsimd.tensor_sub(As, Fm, FmT)
                R3 = F4[:, :, 16:32].rearrange("p c (i j) -> p c i j", i=4)
                # TensorTensor/Reduce are limited to 3 free dims, so KA and
                # its j-reduction run per tile within the block
                KA = small.tile([P, w, 4, 4, 4], F32, name="KA")  # [p,c,i,k,j]
                ka_eng = nc.gpsimd if KA_POOL else nc.vector
                for c in range(w):
                    ka_eng.tensor_mul(
                        KA[:, c],
                        R3[:, c].unsqueeze(2).broadcast_to((P, 4, 4, 4)),
                        R3[:, c].unsqueeze(1).broadcast_to((P, 4, 4, 4)),
                    )
                dtc = small.tile([P, w], F32, name="dtc")
                nc.gpsimd.tensor_scalar(out=dtc, in0=SG[:, :, 0],
                                        scalar1=DT_MAX - DT_MIN, scalar2=DT_MIN,
                                        op0=OP.mult, op1=OP.add)
                ndtd = small.tile([P, w], F32, name="ndtd")
                nc.gpsimd.tensor_scalar(out=ndtd, in0=SG[:, :, 1],
                                        scalar1=-0.5 * (DT_MAX - DT_MIN),
                                        scalar2=-0.5 * DT_MIN,
                                        op0=OP.mult, op1=OP.add)
                A1 = small.tile([P, w, 4, 4], F32, name="A1")
                nc.gpsimd.tensor_mul(
                    A1, As, dtc.unsqueeze(2).unsqueeze(3).broadcast_to((P, w, 4, 4)))
                Kf = small.tile([P, w, 4, 4], F32, name="Kf")  # K[i,k]
                for c in range(w):
                    nc.vector.tensor_reduce(out=Kf[:, c], in_=KA[:, c],
                                            axis=AX.X, op=OP.add)
                bl.update(SG=SG, A1=A1, Kf=Kf, ndtd=ndtd)

            def emit_chain_b(B):
                """Chain second half -> Dm16 [p, c, i, j] fp16."""
                w = widths[B]
                bl = blocks[B]
                F4, SG = bl.pop("F4"), bl.pop("SG")
                Kf, A1, ndtd = bl.pop("Kf"), bl.pop("A1"), bl.pop("ndtd")
                Ks = small.tile([P, w, 4, 4], F32, name="Ks")
                nc.gpsimd.tensor_mul(
                    Ks, Kf, ndtd.unsqueeze(2).unsqueeze(3).broadcast_to((P, w, 4, 4)))
                A = small.tile([P, w, 4, 4], F32, name="A")   # A[p,c,i,j]
                nc.gpsimd.tensor_add(A, Ks, A1)
                wr = SG[:, :, 2:6]
                ww = F4[:, :, 38:42]
                UBt = small.tile([P, w, 4, 4], F32, name="UBt")  # [p,c,j,i]
                nc.gpsimd.tensor_mul(
                    UBt,
                    A.rearrange("p c i j -> p c j i"),
                    wr.unsqueeze(2).broadcast_to((P, w, 4, 4)),
                )
                usum = small.tile([P, w, 4], F32, name="usum")
                nc.vector.tensor_reduce(out=usum, in_=UBt, axis=AX.X, op=OP.add)
                u = small.tile([P, w, 4], F32, name="u")
                nc.gpsimd.tensor_add(u, usum, wr)
                W16 = small.tile([P, w, 4, 4], F32, name="W16")
                nc.gpsimd.tensor_mul(
                    W16,
                    ww.unsqueeze(3).broadcast_to((P, w, 4, 4)),
                    u.unsqueeze(2).broadcast_to((P, w, 4, 4)),
                )
                Dm16 = small.tile([P, w, 4, 4], F16, name="Dm16")
                nc.vector.tensor_add(Dm16, A, W16)
                bl["Dm16"] = Dm16

            def emit_bd(t):
                """Permute D to (j,u)-partition layout on the PE (16 tiny
                matmuls vs identity slices), then build the fp8 block-diag
                mixing lhsT (mask + 256x scale) on gpsimd."""
                B, c, w = tile2blk[t]
                Dm16 = blocks[B]["Dm16"]
                if c == w - 1:
                    del blocks[B]
                v = v_ps.tile([P, 16], F32, name="v", tag="v")  # v[32j+u, 4g+i]
                # PSUM AP base partitions are limited to {0,32,64}: the j=3
                # quadrant is written by a 64-wide matmul based at 64 whose
                # zero-padded lhsT writes zeros into [64:96); the j=2 matmuls
                # come after and overwrite that region.
                for g in range(4):
                    nc.tensor.matmul(
                        v[64:128, 4 * g:4 * g + 4],
                        lhsT=eyp_s[:, g, :],
                        rhs=Dm16[:, c, :, 3],
                        start=True, stop=True, skip_group_check=True)
                for j in range(3):
                    for g in range(4):
                        nc.tensor.matmul(
                            v[32 * j:32 * j + 32, 4 * g:4 * g + 4],
                            lhsT=eye_s[:, 32 * g:32 * g + 32],
                            rhs=Dm16[:, c, :, j],
                            start=True, stop=True, skip_group_check=True)
                vs = small.tile([P, 16], F32, name="vs")
                nc.scalar.copy(out=vs, in_=v)
                # walrus limits TensorScalarPtr APs to partition + 2 free
                # dims, so build bd as [p, (g i), u']
                bd = bdp.tile([P, 16, 32], F8, name="bd")  # [p, (g,i), u']
                nc.gpsimd.scalar_tensor_tensor(
                    out=bd,
                    in0=vs.unsqueeze(2).broadcast_to((P, 16, 32)),
                    scalar=1.0,
                    in1=um_s.unsqueeze(1).broadcast_to((P, 16, 32)),
                    op0=OP.mult, op1=OP.mult)
                state[t] = {"bd": bd}

            def emit_mix(t):
                """Block-diagonal mixing matmuls + drains for tile t."""
                bd = state.pop(t)["bd"]
                x_t = x_tiles.pop(t)
                dlt = dp.tile([P, D], F8, name="dlt")
                for g in range(NSTR):
                    mx = mx_ps.tile([P, BS], F32, tag="mx")
                    nc.tensor.matmul(mx,
                                     lhsT=bd[:, 4 * g:4 * g + 4, :].rearrange("p i u -> p (i u)"),
                                     rhs=x_t[:, g * BS:(g + 1) * BS],
                                     start=True, stop=True)
                    sl = slice(g * BS, (g + 1) * BS)
                    if g in (0, 1):
                        nc.vector.tensor_scalar_mul(dlt[:, sl], mx, 1.0)
                    else:
                        nc.scalar.copy(out=dlt[:, sl], in_=mx)
                dlts[t] = dlt

            def emit_store(t):
                nc.scalar.dma_start(out=dlt_out[t * P:(t + 1) * P, :],
                                    in_=dlts.pop(t))

            dlts = {}
            # schedule: proj(t) at step t; chain_a(B) one step after the
            # block's last proj, chain_b next step; bd(t) >= chain_b+1
            # (one per step); mix(t) and store(t) follow one step apart.
            # Stage emit order within a step: chain first, then bd, proj,
            # mix, store (empirically best - "y" order).
            from collections import defaultdict as _dd
            sched = _dd(list)
            for B, (s0, w) in enumerate(zip(starts, widths)):
                for c in range(w):
                    sched[s0 + c].append((4, "proj", s0 + c))
                sched[s0 + w].append((1, "cha", B))
                sched[s0 + w + 1].append((2, "chb", B))
                for c in range(w):
                    sched[s0 + w + 2 + c].append((3, "bd", s0 + c))
                    sched[s0 + w + 3 + c].append((5, "mix", s0 + c))
                    sched[s0 + w + 4 + c].append((6, "store", s0 + c))
            fns = {"proj": emit_proj, "cha": emit_chain_a, "chb": emit_chain_b,
                   "bd": emit_bd, "mix": emit_mix, "store": emit_store}
            last_step = max(sched)
            for t in range(last_step + 1):
                load_x(t + PF)
                if t % MEGA == 0:
                    load_xt(t // MEGA + 2)
                for _, stg, arg in sorted(sched[t]):
                    fns[stg](arg)
                if t < nt:
                    warm(WARM_TILE)

    nc.finalize()
    return nc


def prep_consts(inputs):
    """Pack the 42 projection rows + per-feature constants (host side)."""
    Wall = np.zeros((NF, D), np.float32)
    Wall[0:16] = np.asarray(inputs["W_conv"], np.float32)
    Wall[16:32] = np.asarray(inputs["W_diss"], np.float32)
    Wall[32] = np.asarray(inputs["W_dt_c"], np.float32)[0]
    Wall[33] = np.asarray(inputs["W_dt_d"], np.float32)[0]
    a_r = float(np.asarray(inputs["alpha_read_in"])[0])
    a_w = float(np.asarray(inputs["alpha_write_out"])[0])
    Wall[34:38] = a_r * np.asarray(inputs["W_read"], np.float32)
    Wall[38:42] = a_w * np.asarray(inputs["W_write"], np.float32)

    C = np.zeros((NF,), np.float32)
    C[0:16] = np.asarray(inputs["conserv_A"], np.float32)[0].reshape(16) + np.asarray(
        inputs["b_conv"], np.float32)
    C[16:32] = np.asarray(inputs["diss_A"], np.float32)[0].reshape(16) + np.asarray(
        inputs["b_diss"], np.float32)
    C[32] = float(np.asarray(inputs["log_dt_c"])[0, 0]) + float(
        np.asarray(inputs["b_dt_c"])[0])
    C[33] = float(np.asarray(inputs["log_dt_d"])[0, 0]) + float(
        np.asarray(inputs["b_dt_d"])[0])
    C[34:38] = np.asarray(inputs["read_in"], np.float32).reshape(4)
    C[38:42] = np.asarray(inputs["write_out"], np.float32).reshape(4)

    # wt[p, k, f] = WSCALE * Wall[f, k*128 + p], flattened to [128, 16*42]
    wt = np.ascontiguousarray(
        (WSCALE * Wall).T.reshape(NBLK, P, NF).transpose(1, 0, 2).reshape(P, NBLK * NF)
    ).astype(NP_F8)
    cv = np.ascontiguousarray(np.broadcast_to(C[None, :], (P, NF))).astype(np.float32)
    eye = np.eye(P, dtype=np.float16)
    # eyp[tok, g, 32+u'] = (tok == 32g+u'), zero-padded for the j=3 write
    eyp = np.zeros((P, 4, 64), np.float16)
    for g in range(4):
        eyp[32 * g:32 * g + 32, g, 32:64] = np.eye(32, dtype=np.float16)
    eyp = eyp.reshape(P, 4 * 64)
    # um[32j+u, u'] = DSC * (u' == u)
    um = DSC * np.tile(np.eye(32, dtype=np.float32), (4, 1))
    iv = np.full((P, 1), 1.0 / WSCALE, np.float32)
    cf = np.ascontiguousarray(np.concatenate([cv, iv, um], axis=1))
    ch = np.ascontiguousarray(np.concatenate([eye, eyp], axis=1).astype(np.float16))
    return wt, cf, ch


def kernel(**inputs):
    global LAST_RESULTS
    x = np.asarray(inputs["x"], np.float32)
    B, T, _ = x.shape
    tok = B * T
    tpc = tok // NCORES
    nt = tpc // P
    xf = np.ascontiguousarray(x.reshape(tok, D))

    wt, cf, ch = prep_consts(inputs)

    if tpc not in _NC_CACHE:
        _NC_CACHE[tpc] = build_nc(tpc)
    nc = _NC_CACHE[tpc]

    in_maps = []
    for c in range(NCORES):
        xc = xf[c * tpc:(c + 1) * tpc]
        # mixing rhs: per 128-token tile, partitions are (j, u) pairs per
        # 32-token group: xh[128T + 32j+u, 512g + d] = x[128T+32g+u, 512j + d]
        xh = np.ascontiguousarray(
            xc.reshape(nt, 4, 32, 4, BS).transpose(0, 3, 2, 1, 4).reshape(tpc, D)
        ).astype(NP_F8)
        # projection lhsT: d-major, RMS scale folded in
        s = (1.0 / np.sqrt(np.mean(xc.astype(np.float64) ** 2, axis=1) + EPS)
             ).astype(np.float32)
        xn = xc * s[:, None]
        xt = np.ascontiguousarray(
            xn.T.reshape(NBLK, P, tpc).transpose(1, 0, 2)).astype(NP_F8)
        in_maps.append({"xh": xh, "xt": xt, "wt": wt, "cf": cf, "ch": ch})
    names = {t.name for t in nc.m.functions[0].inputs} if hasattr(nc.m.functions[0], "inputs") else None
    if names:
        in_maps = [{k: v for k, v in m.items() if k in names} for m in in_maps]

    res = run_bass_kernel_spmd(nc, in_maps, core_ids=list(range(NCORES)), trace=TRACE)
    LAST_RESULTS = res

    out = np.empty((tok, D), np.float32)
    for c in range(NCORES):
        xc = xf[c * tpc:(c + 1) * tpc]
        # un-permute: dlt[128T + 32i+u, 512g + d] = 256*delta[128T+32g+u, 512i+d]
        dl = res.results[c]["dlt"].astype(np.float32) * (1.0 / DSC)
        dl = dl.reshape(nt, 4, 32, 4, BS).transpose(0, 3, 2, 1, 4).reshape(tpc, D)
        out[c * tpc:(c + 1) * tpc] = xc + dl
    return out.reshape(B, T, D)
